# revision 1
# baseline (speedup 1.0000x reference)
"""Trainium2 Bass kernel for nn_ATConv (dynamic per-sample depthwise 3x3 conv).

Contract: kernel(**inputs) takes the FULL unsharded inputs (numpy arrays, keys
as in setup_inputs()) and returns the FULL output [16, 256, 96, 96] float32.
Internally shards batch across 8 NeuronCores (2 samples/core), runs one SPMD
Bass program via run_bass_kernel_spmd, and gathers.

Math (per sample):
    kp     = W_kp @ x + b_kp                  (1x1 conv)
    pooled = adaptive_avg_pool(kp, 9 bins)    = W_kp @ avgpool(x) + b_kp
    ker    = gelu_tanh(pooled) @ W_kg^T + b_kg
    kf     = ker - sigmoid(diff_ctrl) * mean(ker)
    xp     = W_x @ x + b_x
    y      = depthwise3x3(xp, kf)             (zero padding)
    out    = W_p @ y + b_p

Device mapping (per core, 2 samples, phases emitted A0 B0 A1 B1 C0 C1 so
sample 1's load/xp pass overlaps sample 0's depthwise/projection):
    - A: DMA x; DVE streams each pool bin once producing the fp16 copy of x
      (matmul operand) + fp32 bin sums (adaptive avg pool) via accum_out;
      PE xp-pass matmuls (fp16, 1 cyc/row); ACT(+DVE) fuse the bias add into
      the PSUM->SBUF copy writing a zero-padded 98x98 fp16 xp plane.
    - B (tiny): W_kp matmul fp32 on PE, gelu-tanh composed via Sigmoid on
      ACT, 9->9 linear + lateral inhibition as fused DVE ops; PE-tap weights
      = row-scaled W_p^T on DVE.
    - C: depthwise taps split across DVE (product 4x + add 2x fp16 ops),
      GPSIMD (fused MAC), and PE (tap folded into the output projection as a
      matmul against a shifted window of the padded xp, accumulated in the
      same PSUM group as W_p @ y); ACT bias-copy PSUM->SBUF; DMA out.

Scheduling quirks handled explicitly (walrus allows only ONE sync-wait per
Activation instruction; other engines are also wait-limited):
    - all f32 consts arrive in ONE SWDGE DMA ("bias4"+"wkgb" packs), the
      fp16 weights in one more, wkpT in one more -> few DMA clock lanes;
    - dedicated "clock touch" ops advance each engine's view of those lanes
      once, so real ops carry at most one fresh wait;
    - xp pad stripes are zeroed once in warmup slots (pads are never
      overwritten, pool slots recycle), avoiding per-sample cross-engine
      write-write deps on the xp planes.
"""

import os
import sys
from contextlib import ExitStack

import numpy as np

if "/opt/trn_rl_repo" not in sys.path:
    sys.path.insert(0, "/opt/trn_rl_repo")

import concourse.bass as bass
import concourse.tile as tile
from concourse import bacc, mybir
from concourse._compat import with_exitstack
from concourse.bass_utils import run_bass_kernel_spmd

# ---- problem dims (hardcoded per harness contract) ----
B, C, H, W = 16, 256, 96, 96
K2 = 9
HW = H * W                # 9216
NCORES = 8
SPC = B // NCORES         # samples per core = 2
CT = C // 128             # channel tiles = 2
BIN = HW // K2            # 1024 (adaptive pool bin)
XCH = 3 * BIN             # 3072 x-load chunk (3 bins, 8 xp chunks)
NXCH = HW // XCH          # 3
BLK_R = int(os.environ.get("ATC_BLKR", "16"))  # y block rows
NBLK = H // BLK_R         # 6
OCH_R = 4                 # row chunk for matmul free dim (4*96=384 <= 512 fp32)
OCH = OCH_R * W           # 384
NOCH = H // OCH_R         # 24 chunks per plane
CPB = BLK_R // OCH_R      # out chunks per block = 4
PW = W + 2                # padded row width (zero border for shifted windows)
PHW = PW * (H + 2)        # padded plane size 98*98

F32 = mybir.dt.float32
F16 = mybir.dt.float16

# ---- tuning knobs ----
MM_DT_NAME = os.environ.get("ATC_MM_DT", "f16")   # f16 | f32
MM_DT = {"f16": F16, "f32": F32}[MM_DT_NAME]
MM_NP = {"f16": np.float16, "f32": np.float32}[MM_DT_NAME]

def _taplist(env, default):
    v = os.environ.get(env, default)
    return tuple(int(t) for t in v.split(",") if t != "")

# taps handled by DVE / GPSIMD (rest folded into PE output projection);
# center tap (4) must be on DVE (it initializes y).
# fp16 alignment: dj==1 taps (1,7) are 4B-misaligned -> prefer them on PE.
DVE_TAPS = _taplist("ATC_DVE_TAPS", "0,2,4,6,8")
GPS_TAPS = _taplist("ATC_GPS_TAPS", "")
PE_TAPS = tuple(t for t in range(K2) if t not in DVE_TAPS and t not in GPS_TAPS)
assert 4 in DVE_TAPS
TAP_SPLIT = os.environ.get("ATC_TAP_SPLIT", "1") == "1"  # DVE taps as mul+add
XPC_MOD = int(os.environ.get("ATC_XPC_MOD", "6"))  # every Nth xp-copy on DVE (0=off)
OUTC_ENG = os.environ.get("ATC_OUTC", "act")       # out-copy engine: act|dve
# optional half-taps: "t:k,t:k" puts PE tap t for channel-tile k on DVE
HALF_TAP = os.environ.get("ATC_HALF_TAP", "3:0")
HALF_TK = tuple(
    tuple(int(v) for v in e.split(":")) for e in HALF_TAP.split(",") if e
)
# tail taper: for the last TAPER_N blocks of the last sample, move PE taps
# to DVE so both engines drain together instead of PE finishing alone
TAPER_N = int(os.environ.get("ATC_TAPER", "1"))
TAPER_TAPS = _taplist("ATC_TAPER_TAPS", "1,5")  # PE taps moved to DVE in taper
# taps whose products run on ACT (adds stay on DVE); subset of DVE_TAPS
ACT_PROD = _taplist("ATC_ACT_PROD", "2,6")
# gelu_tanh(x) = x * sigmoid(GELU_C * (x + 0.044715 x^3))
GELU_C = 1.5957691216057308

AF = mybir.ActivationFunctionType
ALU = mybir.AluOpType


@with_exitstack
def _atconv_kernel(ctx: ExitStack, tc: "tile.TileContext"):
    nc = tc.nc

    # x arrives pre-cast to fp16 (host): halves DMA-in and the loaded tile
    # is directly the matmul operand; pooling accumulates fp32 from it.
    x_d = nc.dram_tensor("x", [SPC, C, H, W], MM_DT, kind="ExternalInput").ap()
    # packed weights: w16 = [wxT ; wpT] (fp16), wkpT fp32
    w16_d = nc.dram_tensor("w16", [2 * C, C], MM_DT, kind="ExternalInput")
    wkpT_d = nc.dram_tensor("wkpT", [C, C], F32, kind="ExternalInput")
    # bias4 rows: bx, bkp, bp, fac(=sigmoid(diff_ctrl))
    bias4_d = nc.dram_tensor("bias4", [4, C], F32, kind="ExternalInput")
    # wkgb rows 0..8 = wkgT (= w_kg.T), row 9 = bkg
    wkgb_d = nc.dram_tensor("wkgb", [K2 + 1, K2], F32, kind="ExternalInput")
    out_d = nc.dram_tensor("out", [SPC, C, H, W], F32, kind="ExternalOutput").ap()

    consts = ctx.enter_context(tc.tile_pool(name="consts", bufs=1))
    xch_pool = ctx.enter_context(tc.tile_pool(name="xch", bufs=int(os.environ.get("ATC_XCHB", "6"))))
    xp_pool = ctx.enter_context(tc.tile_pool(name="xppool", bufs=2 * CT))
    y_pool = ctx.enter_context(tc.tile_pool(name="ypool", bufs=int(os.environ.get("ATC_YB", "8"))))
    stage_pool = ctx.enter_context(tc.tile_pool(name="stage", bufs=int(os.environ.get("ATC_STGB", "4"))))
    small = ctx.enter_context(tc.tile_pool(name="small", bufs=2 * CT))
    kf_pool = ctx.enter_context(tc.tile_pool(name="kfpool", bufs=2 * CT))
    wtap_pool = ctx.enter_context(tc.tile_pool(name="wtap", bufs=2))
    scr_pool = ctx.enter_context(tc.tile_pool(name="tapscr", bufs=int(os.environ.get("ATC_SCRB", "4"))))
    psum4 = os.environ.get("ATC_PSUM4", "1") == "1"
    mm_pool = ctx.enter_context(
        tc.tile_pool(name="mmps", bufs=4 if psum4 else 3, space="PSUM")
    )
    sps_pool = mm_pool if psum4 else ctx.enter_context(
        tc.tile_pool(name="sps", bufs=2, space="PSUM")
    )

    # ---------------- constants + engine-clock priming ----------------
    # Dependency-free first ACT op: absorbs the implicit ACT table-load (for
    # the sigmoid set) so it doesn't consume a real op's sync-wait slot.
    actprime = consts.tile([128, 1], F32, name="actprime")
    nc.scalar.activation(out=actprime, in_=actprime, func=AF.Sigmoid,
                         bias=0.0, scale=0.0)  # scale=0: input not read

    w16_sb = consts.tile([128, 2 * CT, C], MM_DT, name="w16sb")
    nc.sync.dma_start(
        out=w16_sb, in_=bass.AP(w16_d, 0, [[C, 128], [128 * C, 2 * CT], [1, C]])
    )
    wkp_sb = consts.tile([128, CT, C], F32, name="wkpsb")
    nc.sync.dma_start(
        out=wkp_sb, in_=bass.AP(wkpT_d, 0, [[C, 128], [128 * C, CT], [1, C]])
    )
    ball = consts.tile([128, 4, CT], F32, name="ball")
    nc.gpsimd.dma_start(
        out=ball, in_=bass.AP(bias4_d, 0, [[1, 128], [C, 4], [128, CT]])
    )
    wkgall = consts.tile([128, K2 + 1, K2], F32, name="wkgall")
    nc.gpsimd.dma_start(
        out=wkgall, in_=bass.AP(wkgb_d, 0, [[0, 128], [K2, K2 + 1], [1, K2]])
    )

    w_sb = {}
    for k in range(CT):
        w_sb["wx", k] = w16_sb[:, k, :]
        w_sb["wp", k] = w16_sb[:, CT + k, :]
        w_sb["wkp", k] = wkp_sb[:, k, :]
    bias_sb = {}
    for r, nm in enumerate(("bx", "bkp", "bp", "fac")):
        for i in range(CT):
            bias_sb[nm, i] = ball[:, r, i:i + 1]
    wkg_bc = wkgall[:, 0:K2, :]
    bkg_bc = wkgall[:, K2, :]

    # ACT clock touches: one per const-DMA lane (distinct outputs: avoid WAW)
    tch1 = consts.tile([128, 1], F32, name="tch1")
    nc.scalar.copy(tch1, ball[:, 0, 0:1])
    tch2 = consts.tile([128, 1], F32, name="tch2")
    nc.scalar.copy(tch2, wkgall[:, 0, 0:1])

    # PE clock touches for the two weight-DMA lanes (tiny K=1 matmuls)
    for idx, wt in enumerate((w16_sb, wkp_sb)):
        pp = sps_pool.tile([1, 16], F32, name="peprime", tag="mmps" if psum4 else "plps")
        nc.tensor.matmul(pp, lhsT=wt[0:1, 0, 0:1], rhs=wt[0:1, 0, 0:16],
                         start=True, stop=True)

    st = [dict() for _ in range(SPC)]  # per-sample state

    # ------------- phase A: load x, pool-and-cast, xp pass -------------
    def phase_A_init(s):
        xpool = {}
        for k in range(CT):
            xpool[k] = small.tile([128, K2], F32, name="xpool", tag="xpool")
        # xp stored as a zero-padded 98x98 plane so every shifted 3x3 window
        # read (DVE taps and PE tap-matmuls) is a full, uncropped window.
        xp3 = {}
        last_pad = None
        for k in range(CT):
            xpt = xp_pool.tile([128, PHW], MM_DT, name="xp", tag="xp")
            nc.vector.memset(xpt[:, 0:PW], 0.0)                       # top row
            nc.vector.memset(xpt[:, PHW - PW:PHW], 0.0)               # bottom row
            nc.vector.memset(bass.AP(xpt.tensor, xpt.offset + W + 1,
                                     [xpt.ap[0], [PW, H + 1], [1, 2]]), 0.0)
            xp3[k] = xpt.rearrange("p (r w) -> p r w", w=PW)
            last_pad = xpt
        # ACT observes the pad memsets once, so the xp interior copies below
        # don't each carry a cross-engine write-write wait (1-wait limit).
        tchp = small.tile([128, 1], F32, name="tchp", tag="tchp", bufs=SPC)
        nc.scalar.copy(tchp, last_pad[:, 0:1])
        st[s]["xpool"] = xpool
        st[s]["xp3"] = xp3
        st[s]["ncopy"] = 0

    def phase_A_xc(s, xc):
        xpool, xp3 = st[s]["xpool"], st[s]["xp3"]
        x16s = []
        for k in range(CT):
            t = xch_pool.tile([128, XCH], MM_DT, name="xcht", tag="xcht")
            nc.sync.dma_start(
                out=t,
                in_=x_d[s, k * 128:(k + 1) * 128]
                .rearrange("c h w -> c (h w)")[:, xc * XCH:(xc + 1) * XCH],
            )
            # in-place identity stream per bin: fp32 bin sum lands in
            # accum_out (adaptive avg pool), data unchanged (4x mode)
            for bb in range(XCH // BIN):
                nc.vector.tensor_scalar(
                    t[:, bb * BIN:(bb + 1) * BIN],
                    t[:, bb * BIN:(bb + 1) * BIN],
                    1.0, None, ALU.mult, ALU.add,
                    accum_out=xpool[k][:, 3 * xc + bb:3 * xc + bb + 1],
                )
            x16s.append(t)
        # pairs of 4-row chunks share one 2-bank PSUM tile (independent
        # accumulation groups, bank-aligned halves) so ONE copy moves
        # 8 rows -> halves the per-copy fixed overhead
        for pp in range(XCH // OCH // 2):
            cc0 = xc * (XCH // OCH) + 2 * pp  # first 4-row chunk of pair
            for o in range(CT):
                ps = mm_pool.tile([128, 1024], F32, name="psxp", tag="mmps")
                for half in range(2):
                    nn = 2 * pp + half
                    for k in range(CT):
                        nc.tensor.matmul(
                            ps[:, half * 512:half * 512 + OCH],
                            lhsT=w_sb["wx", k][:, o * 128:(o + 1) * 128],
                            rhs=x16s[k][:, nn * OCH:(nn + 1) * OCH],
                            start=(k == 0),
                            stop=(k == CT - 1),
                        )
                src = ps.rearrange("p (h x) -> p h x", x=512)[:, :, 0:OCH]
                dst = xp3[o][:, 1 + OCH_R * cc0:1 + OCH_R * (cc0 + 2), 1:W + 1]
                st[s]["ncopy"] += 1
                if XPC_MOD and st[s]["ncopy"] % XPC_MOD == 0:
                    nc.vector.tensor_scalar(
                        dst, src, bias_sb["bx", o], None, ALU.add
                    )
                else:
                    nc.scalar.activation(
                        out=dst, in_=src, func=AF.Identity,
                        bias=bias_sb["bx", o], scale=1.0,
                    )

    # ------------- phase B: kernel synthesis (fp32 path) -------------
    def phase_B(s):
        xpool = st[s]["xpool"]
        kf = {}
        for i in range(CT):
            pl_ps = sps_pool.tile([128, K2], F32, name="plps", tag="mmps" if psum4 else "plps")
            for k in range(CT):
                nc.tensor.matmul(
                    pl_ps,
                    lhsT=w_sb["wkp", k][:, i * 128:(i + 1) * 128],
                    rhs=xpool[k],
                    start=(k == 0),
                    stop=(k == CT - 1),
                )
            # px = pooled (bin sums / BIN + bias); gelu-tanh via sigmoid
            px = small.tile([128, K2], F32, name="px", tag="px")
            nc.scalar.activation(
                out=px, in_=pl_ps, func=AF.Identity,
                bias=bias_sb["bkp", i], scale=1.0 / BIN,
            )
            px2 = small.tile([128, K2], F32, name="px2", tag="px2")
            nc.scalar.activation(out=px2, in_=px, func=AF.Square)
            t1 = small.tile([128, K2], F32, name="gt1", tag="gt1")
            nc.vector.tensor_scalar(t1, px2, 0.044715, 1.0, ALU.mult, ALU.add)
            gz = small.tile([128, K2], F32, name="gz", tag="gz")
            nc.vector.tensor_mul(gz, px, t1)
            sg = small.tile([128, K2], F32, name="sg", tag="sg")
            nc.scalar.activation(out=sg, in_=gz, func=AF.Sigmoid, bias=0.0, scale=GELU_C)
            pooled_g = small.tile([128, K2], F32, name="pooledg", tag="pooledg")
            nc.vector.tensor_mul(pooled_g, px, sg)
            # ker = sum_k pooled_g[:,k] * wkgT[k,:] + bkg  (fused DVE MACs)
            kacc = small.tile([128, K2], F32, name="kacc", tag="kacc")
            nc.vector.scalar_tensor_tensor(
                out=kacc, in0=wkg_bc[:, 0, :], scalar=pooled_g[:, 0:1],
                in1=bkg_bc, op0=ALU.mult, op1=ALU.add,
            )
            for kk in range(1, K2):
                nc.vector.scalar_tensor_tensor(
                    out=kacc, in0=wkg_bc[:, kk, :], scalar=pooled_g[:, kk:kk + 1],
                    in1=kacc, op0=ALU.mult, op1=ALU.add,
                )
            kmean = small.tile([128, 1], F32, name="kmean", tag="kmean")
            nc.vector.tensor_reduce(
                out=kmean, in_=kacc, axis=mybir.AxisListType.X, op=ALU.add
            )
            ktmp = small.tile([128, 1], F32, name="ktmp", tag="ktmp")
            nc.vector.tensor_scalar(
                ktmp, kmean, bias_sb["fac", i], 1.0 / K2, ALU.mult, ALU.mult
            )
            kf[i] = kf_pool.tile([128, K2], F32, name="kft", tag="kft")
            nc.vector.tensor_scalar(kf[i], kacc, ktmp[:, 0:1], None, ALU.subtract)

        # PE-tap weights: wtap[t,k] = wpT[k] * kf[k][:,t]  (row-scaled)
        wtap = {}
        for t in PE_TAPS:
            for k in range(CT):
                if (t, k) in HALF_TK:
                    continue
                wt = wtap_pool.tile([128, C], MM_DT, name="wtapt", tag=f"wt{t}_{k}")
                nc.vector.tensor_scalar(
                    wt, w_sb["wp", k], kf[k][:, t:t + 1], None, ALU.mult
                )
                wtap[t, k] = wt
        st[s]["kf"] = kf
        st[s]["wtap"] = wtap

    # ---- phase C: depthwise taps (DVE/GPSIMD) + out projection (PE) ----
    # tap t=(di,dj): y[i,j] += kf[t] * xp_pad[i+di, j+dj]  (padded coords)
    def phase_C_block(s, b):
        xp3, kf, wtap = st[s]["xp3"], st[s]["kf"], st[s]["wtap"]
        if True:
            r0 = b * BLK_R
            taper = s == SPC - 1 and b >= NBLK - TAPER_N
            extra = [t for t in TAPER_TAPS if t in PE_TAPS] if taper else []
            yb = {}
            for k in range(CT):
                yt = y_pool.tile([128, BLK_R * W], MM_DT, name="yblk", tag="yblk")
                # center tap initializes the whole block (tensor_scalar, 4x)
                nc.vector.tensor_scalar(
                    yt,
                    xp3[k][:, r0 + 1:r0 + 1 + BLK_R, 1:W + 1],
                    kf[k][:, 4:5],
                    None,
                    ALU.mult,
                )
                dtaps = [t for t in DVE_TAPS if t != 4] + extra
                for ht, hk in HALF_TK:
                    if k == hk and ht not in dtaps:
                        dtaps.append(ht)
                for t in dtaps:
                    di, dj = t // 3, t % 3
                    src = xp3[k][:, r0 + di:r0 + di + BLK_R, dj:dj + W]
                    if TAP_SPLIT:
                        tmp = scr_pool.tile(
                            [128, BLK_R * W], MM_DT, name="tapscr", tag="tapscr"
                        )
                        if t in ACT_PROD:
                            nc.scalar.activation(
                                out=tmp, in_=src, func=AF.Copy,
                                bias=0.0, scale=kf[k][:, t:t + 1],
                            )
                        else:
                            nc.vector.tensor_scalar(
                                tmp, src, kf[k][:, t:t + 1], None, ALU.mult
                            )
                        nc.vector.tensor_add(yt, yt, tmp)
                    else:
                        nc.vector.scalar_tensor_tensor(
                            out=yt, in0=src, scalar=kf[k][:, t:t + 1],
                            in1=yt, op0=ALU.mult, op1=ALU.add,
                        )
                for t in GPS_TAPS:
                    di, dj = t // 3, t % 3
                    nc.gpsimd.scalar_tensor_tensor(
                        out=yt,
                        in0=xp3[k][:, r0 + di:r0 + di + BLK_R, dj:dj + W],
                        scalar=kf[k][:, t:t + 1],
                        in1=yt,
                        op0=ALU.mult,
                        op1=ALU.add,
                    )
                yb[k] = yt

            pe_tk = [
                (t, k)
                for t in PE_TAPS
                for k in range(CT)
                if (t, k) not in HALF_TK and t not in extra
            ]
            n_mm = CT + len(pe_tk)
            for o in range(CT):
                stg = stage_pool.tile([128, CPB * OCH], F32, name="stg", tag="stg")
                for pp in range(CPB // 2):
                    ps = mm_pool.tile([128, 1024], F32, name="psout", tag="mmps")
                    for half in range(2):
                        cc = 2 * pp + half
                        m0 = r0 + cc * OCH_R  # first row of this out chunk
                        psv = ps[:, half * 512:half * 512 + OCH]
                        # tap-MMs first: they depend only on xp, so PE can
                        # run ahead of the DVE taps across open PSUM groups;
                        # the y-MMs (gated on DVE) close each group.
                        mm_i = 0
                        for t, k in pe_tk:
                            di, dj = t // 3, t % 3
                            nc.tensor.matmul(
                                psv,
                                lhsT=wtap[t, k][:, o * 128:(o + 1) * 128],
                                rhs=xp3[k][:, m0 + di:m0 + di + OCH_R, dj:dj + W],
                                start=(mm_i == 0),
                                stop=(mm_i == n_mm - 1),
                            )
                            mm_i += 1
                        for k in range(CT):
                            nc.tensor.matmul(
                                psv,
                                lhsT=w_sb["wp", k][:, o * 128:(o + 1) * 128],
                                rhs=yb[k][:, cc * OCH:(cc + 1) * OCH],
                                start=(mm_i == 0),
                                stop=(mm_i == n_mm - 1),
                            )
                            mm_i += 1
                    src = ps.rearrange("p (h x) -> p h x", x=512)[:, :, 0:OCH]
                    dst = stg[:, 2 * pp * OCH:2 * (pp + 1) * OCH]
                    if OUTC_ENG == "dve":
                        nc.vector.tensor_scalar(
                            dst, src, bias_sb["bp", o], None, ALU.add
                        )
                    else:
                        nc.scalar.activation(
                            out=dst, in_=src, func=AF.Identity,
                            bias=bias_sb["bp", o], scale=1.0,
                        )
                _store_eng = nc.scalar if os.environ.get("ATC_STORE_ACT", "0") == "1" else nc.sync
                _store_eng.dma_start(
                    out=out_d[s, o * 128:(o + 1) * 128]
                    .rearrange("c h w -> c (h w)")[:, r0 * W:(r0 + BLK_R) * W],
                    in_=stg,
                )

    # Optional on-device repeat loop (ATC_LOOP=R) for timing: reruns the
    # whole per-core computation R times inside one NEFF.
    def _emit_phases():
        phase_A_init(0)
        for xc in range(NXCH):
            phase_A_xc(0, xc)
        phase_B(0)
        if SPC > 1:
            phase_A_init(1)
            for xc in range(NXCH):
                phase_A_xc(1, xc)
                phase_C_block(0, xc)
            phase_B(1)
            for b in range(NXCH, NBLK):
                phase_C_block(0, b)
            for b in range(NBLK):
                phase_C_block(1, b)
        else:
            for b in range(NBLK):
                phase_C_block(0, b)

    import contextlib
    loop_r = int(os.environ.get("ATC_LOOP", "1"))
    loop_ctx = tc.For_i(0, loop_r, 1) if loop_r > 1 else contextlib.nullcontext()
    with loop_ctx:
        _emit_phases()


_NC_CACHE = None


def _get_nc():
    global _NC_CACHE
    if _NC_CACHE is None:
        nc = bacc.Bacc("TRN2", target_bir_lowering=False)
        with tile.TileContext(nc) as tc:
            _atconv_kernel(tc)
        nc.compile()
        _NC_CACHE = nc
    return _NC_CACHE


def _make_in_maps(inputs):
    f32 = lambda a: np.ascontiguousarray(np.asarray(a, dtype=np.float32))
    x = np.ascontiguousarray(np.asarray(inputs["x"], dtype=np.float32).astype(MM_NP))
    dc = f32(inputs["diff_ctrl"]).reshape(C)
    fac = (1.0 / (1.0 + np.exp(-dc.astype(np.float64)))).astype(np.float32)
    w16 = np.concatenate(
        [np.asarray(inputs["w_xproj"], np.float32).T,
         np.asarray(inputs["w_proj"], np.float32).T], axis=0
    ).astype(MM_NP)
    bias4 = np.stack(
        [f32(inputs["b_xproj"]), f32(inputs["b_kp"]), f32(inputs["b_proj"]), fac]
    )
    wkgb = np.concatenate(
        [np.asarray(inputs["w_kg"], np.float32).T,
         f32(inputs["b_kg"]).reshape(1, K2)], axis=0
    ).astype(np.float32)
    base = {
        "w16": np.ascontiguousarray(w16),
        "wkpT": f32(np.asarray(inputs["w_kp"], np.float32).T),
        "bias4": np.ascontiguousarray(bias4),
        "wkgb": np.ascontiguousarray(wkgb),
    }
    return [dict(base, x=x[c * SPC:(c + 1) * SPC]) for c in range(NCORES)]


def run(inputs, trace=False):
    """Returns (out, BassKernelResults)."""
    in_maps = _make_in_maps(inputs)
    res = run_bass_kernel_spmd(
        _get_nc(), in_maps, core_ids=list(range(NCORES)), trace=trace
    )
    out = np.concatenate([r["out"] for r in res.results], axis=0)
    return out, res


def kernel(**inputs) -> np.ndarray:
    out, _ = run(inputs)
    return out


def np_ref(inp, s0, s1):
    """float64 numpy reference (tanh-approx gelu, as jax.nn.gelu default)."""
    x = np.asarray(inp["x"])[s0:s1].astype(np.float64)
    wkp, bkp = np.asarray(inp["w_kp"], np.float64), np.asarray(inp["b_kp"], np.float64)
    wx, bx = np.asarray(inp["w_xproj"], np.float64), np.asarray(inp["b_xproj"], np.float64)
    wp, bp = np.asarray(inp["w_proj"], np.float64), np.asarray(inp["b_proj"], np.float64)
    wkg, bkg = np.asarray(inp["w_kg"], np.float64), np.asarray(inp["b_kg"], np.float64)
    fac = 1 / (1 + np.exp(-np.asarray(inp["diff_ctrl"], np.float64).reshape(C)))
    b = x.shape[0]
    kp = np.einsum("bchw,oc->bohw", x, wkp) + bkp[None, :, None, None]
    pooled = kp.reshape(b, C, K2, HW // K2).mean(-1)
    g = 0.5 * pooled * (1 + np.tanh(np.sqrt(2 / np.pi) * (pooled + 0.044715 * pooled**3)))
    ker = np.einsum("bck,jk->bcj", g, wkg) + bkg
    ker = ker - fac[None, :, None] * ker.mean(axis=-1, keepdims=True)
    xp = np.einsum("bchw,oc->bohw", x, wx) + bx[None, :, None, None]
    xpp = np.pad(xp, ((0, 0), (0, 0), (1, 1), (1, 1)))
    y = np.zeros_like(xp)
    for t in range(K2):
        di, dj = t // 3, t % 3
        y += ker[:, :, t, None, None] * xpp[:, :, di:di + H, dj:dj + W]
    return np.einsum("bchw,oc->bohw", y, wp) + bp[None, :, None, None]


def _test_inputs(seed=0):
    rng = np.random.default_rng(seed)
    return {
        "x": rng.standard_normal((B, C, H, W), dtype=np.float32),
        "w_xproj": (rng.standard_normal((C, C)) / np.sqrt(C)).astype(np.float32),
        "b_xproj": rng.standard_normal(C).astype(np.float32) * 0.1,
        "w_proj": (rng.standard_normal((C, C)) / np.sqrt(C)).astype(np.float32),
        "b_proj": rng.standard_normal(C).astype(np.float32) * 0.1,
        "w_kp": (rng.standard_normal((C, C)) / np.sqrt(C)).astype(np.float32),
        "b_kp": rng.standard_normal(C).astype(np.float32) * 0.1,
        "w_kg": (rng.standard_normal((K2, K2)) / 3.0).astype(np.float32),
        "b_kg": rng.standard_normal(K2).astype(np.float32) * 0.1,
        "diff_ctrl": rng.standard_normal((1, 1, 1, C)).astype(np.float32) * 0.5,
    }


if __name__ == "__main__":
    # CoreSim smoke test of core 0 (2 samples) against the numpy reference.
    from concourse.bass_interp import CoreSim

    inputs = _test_inputs()
    nc = _get_nc()
    in_maps = _make_in_maps(inputs)
    sim = CoreSim(nc)
    for name, arr in in_maps[0].items():
        sim.tensor(name)[:] = arr
    sim.simulate()
    got = np.array(sim.tensor("out"))
    want = np_ref(inputs, 0, SPC)
    err = np.abs(got - want)
    denom = np.abs(want).max()
    print("sim modeled time:", sim.time, "ns")
    print("absmax err:", err.max(), "rel:", err.max() / denom)
    print("rms rel:", np.sqrt(((got - want) ** 2).mean()) / np.sqrt((want**2).mean()))

